# revision 9
# baseline (speedup 1.0000x reference)
"""Trainium2 Bass kernel for nn_Attention_12000138625343.

Full multi-head attention layer (B=2, S=2048, E=1024, H=16, hd=64, interleaved
RoPE on q/k, non-causal softmax) run tensor-parallel over 8 NeuronCores:

  - heads sharded 2-per-core (w1 columns / qkv projection sharded),
  - x replicated, passed pre-transposed [E, B*S] in bf16 so the contraction
    dim lands on SBUF partitions,
  - all matmul operands in bf16 (PSUM accumulation stays fp32): halves HBM
    traffic, SBUF footprint, and the A2A payload; max-norm rel err ~1e-3,
  - scores computed transposed [k, q]; the two heads' K=64 score matmuls are
    packed into disjoint PE row-groups, one exp instruction covers both
    heads' [128, 1024] PSUM block,
  - the v projection runs in its fast transposed orientation (N=512) and is
    flipped back to [k, hd] by PE transposes against an identity,
  - attn@v accumulates rolling per k-chunk with a ones-column appended to v
    producing the softmax denominator; the divide uses the single-op DVE
    reciprocal_approx_fast + DRAM-bounce broadcast DMA + DVE multiply,
  - batch-1 qkv projection / batch-0 output projection matmul chains are
    dribbled into the attention k-chunk loop so the in-order PE stream never
    starves the exp pipeline for long,
  - four AllToAlls (one per batch-half, each gated by that half's last
    softmax divide) convert the head sharding of the attention output o^T
    into row sharding; the GpSimd queue carries ONLY collective triggers and
    recv loads (all input loads ride the sync HWDGE queue) so triggers fire
    the moment their sends complete on every core,
  - each core owns 2 x 128 rows of each batch; host reassembles.
"""

import math

import numpy as np

import concourse.bass as bass
import concourse.mybir as mybir
import concourse.tile as tile
from concourse import bacc
from concourse.bass_utils import run_bass_kernel_spmd
from concourse.masks import make_identity

B, S, E, H = 2, 2048, 1024, 16
HD = E // H  # 64
BASE = 10000.0
N_CORES = 8
HPC = H // N_CORES       # heads per core = 2
R = B * S                # 4096 flattened rows
RT = 512                 # rows per r-tile
NEC = E // 128           # 8 e-chunks of 128
QT = 512                 # q columns per q-tile
N_QT = S // QT           # 4 q-tiles per batch
KC = 128                 # k rows per k-chunk
N_KC = S // KC           # 16 k-chunks per batch
RPB = S // N_CORES       # rows per core per batch = 256

F32 = mybir.dt.float32
BF16 = mybir.dt.bfloat16
EXPF = mybir.ActivationFunctionType.Exp
LNF = mybir.ActivationFunctionType.Ln

_COMPILED = {}


def _build_nc():
    nc = bacc.Bacc("TRN2", target_bir_lowering=False, debug=False,
                   num_devices=N_CORES)

    xT = nc.dram_tensor("xT", [E, R], BF16, kind="ExternalInput").ap()
    wqT = nc.dram_tensor("wqT", [E, 128], BF16, kind="ExternalInput").ap()
    wkT = nc.dram_tensor("wkT", [E, 128], BF16, kind="ExternalInput").ap()
    wvT = nc.dram_tensor("wvT", [E, 128], BF16, kind="ExternalInput").ap()
    w2T = nc.dram_tensor("w2T", [E, E], BF16, kind="ExternalInput").ap()
    cosT = nc.dram_tensor("cosT", [128, S], F32, kind="ExternalInput").ap()
    sinT = nc.dram_tensor("sinT", [128, S], F32, kind="ExternalInput").ap()
    p2T = nc.dram_tensor("p2T", [128, 128], BF16, kind="ExternalInput").ap()
    out = nc.dram_tensor("out", [2 * RPB, E], F32, kind="ExternalOutput").ap()

    with tile.TileContext(nc) as tc:
        _emit(tc, nc, xT, wqT, wkT, wvT, w2T, cosT, sinT, p2T, out)
    nc.compile()
    return nc


def _emit(tc, nc, xT, wqT, wkT, wvT, w2T, cosT, sinT, p2T, out):
    import contextlib
    ctx = contextlib.ExitStack()
    consts = ctx.enter_context(tc.tile_pool(name="consts", bufs=1))
    xtp = ctx.enter_context(tc.tile_pool(name="xtp", bufs=2))
    qkp = ctx.enter_context(tc.tile_pool(name="qkp", bufs=1))
    rawp = ctx.enter_context(tc.tile_pool(name="rawp", bufs=2))
    tmpp = ctx.enter_context(tc.tile_pool(name="tmpp", bufs=2))
    vp = ctx.enter_context(tc.tile_pool(name="vp", bufs=1))
    pp = ctx.enter_context(tc.tile_pool(name="pp", bufs=7))
    smallp = ctx.enter_context(tc.tile_pool(name="smallp", bufs=2))
    dramp = ctx.enter_context(tc.tile_pool(name="dramp", bufs=1, space="DRAM"))
    # PSUM budget (8 banks): qkv-shared 2 + sps 2 x 2 + av 2 = 8
    ps_qkv = ctx.enter_context(tc.tile_pool(name="ps_qkv", bufs=2, space="PSUM"))
    ps_sps = ctx.enter_context(tc.tile_pool(name="ps_sps", bufs=2, space="PSUM"))
    ps_av = ctx.enter_context(tc.tile_pool(name="ps_av", bufs=2, space="PSUM"))

    # ---- bulk input loads on the gpsimd SWDGE queue: descriptor-gen is ~1us
    # each and the transfers fan out across the DMA engine pool in parallel
    # (HWDGE queues serialize the data). With xt bufs=8 nothing ever waits
    # in-FIFO, so the collective triggers emitted later are never delayed. ----
    wq_all = consts.tile([128, NEC, 128], BF16, tag="wq", name="wq_all")
    nc.sync.dma_start(out=wq_all[:], in_=wqT.rearrange("(c p) f -> p c f", p=128))

    xts = []
    for rt in range(2 * N_QT):
        t = xtp.tile([128, NEC, RT], BF16, tag="xt", bufs=8, name=f"xt_{rt}")
        nc.gpsimd.dma_start(
            out=t[:],
            in_=xT.rearrange("(c p) r -> p c r", p=128)[:, :, rt * RT:(rt + 1) * RT])
        xts.append(t)
        if rt == 0:
            wk_all = consts.tile([128, NEC, 128], BF16, tag="wk", name="wk_all")
            nc.gpsimd.dma_start(
                out=wk_all[:], in_=wkT.rearrange("(c p) f -> p c f", p=128))
            wv_all = consts.tile([128, NEC, 128], BF16, tag="wv", name="wv_all")
            nc.gpsimd.dma_start(
                out=wv_all[:], in_=wvT.rearrange("(c p) f -> p c f", p=128))
            p2_sb = consts.tile([128, 128], BF16, tag="p2", name="p2_sb")
            nc.gpsimd.dma_start(out=p2_sb[:], in_=p2T[:, :])
        if rt == 1:
            cos_sb = consts.tile([128, S], F32, tag="cos", name="cos_sb")
            nc.gpsimd.dma_start(out=cos_sb[:], in_=cosT[:, :])
            sin_sb = consts.tile([128, S], F32, tag="sin", name="sin_sb")
            nc.gpsimd.dma_start(out=sin_sb[:], in_=sinT[:, :])
    # w2: 2 MB, overlaps the batch-0 qkv/attention stretch
    w2_all = consts.tile([128, NEC, E], BF16, tag="w2", name="w2_all")
    nc.gpsimd.dma_start(out=w2_all[:], in_=w2T.rearrange("(c p) f -> p c f", p=128))

    ones_f32 = consts.tile([128, 64], F32, tag="ones32", name="ones_f32")
    nc.vector.memset(ones_f32[:], 1.0)
    ones_r = consts.tile([1, 64], F32, tag="onesr", name="ones_r")
    nc.vector.tensor_copy(ones_r[:], ones_f32[0:1, 0:64])
    id_sb = consts.tile([128, 128], F32, tag="idm", name="id_sb")
    make_identity(nc, id_sb[:])

    # A2A buffers, one pair per (batch, half): [8 chunks, 128 e-rows, 128 rows]
    # half 0 carries s in [128j, 128j+128) (ready after q-tile 1),
    # half 1 carries s in [1024+128j, ...) (ready after q-tile 3).
    send_d = {(b, hf): dramp.tile([N_CORES, 128, 128], BF16, name=f"send{b}{hf}")
              for b in range(B) for hf in range(2)}
    recv_d = {(b, hf): dramp.tile([N_CORES, 128, 128], BF16, name=f"recv{b}{hf}")
              for b in range(B) for hf in range(2)}

    qT_sb, kT_sb, v_sb = {}, {}, {}

    def qkv_chains(rt):
        """Return a list of closures, each emitting one matmul chain (+ its
        epilogue) for r-tile rt. Callers dribble these between attention
        steps to keep the in-order PE stream dense but never monolithic."""
        b, st = rt // N_QT, (rt % N_QT) * RT
        xt = xts[rt]

        if b not in qT_sb:
            qT_sb[b] = qkp.tile([128, S], BF16, tag=f"qT{b}", name=f"qT{b}")
            kT_sb[b] = qkp.tile([128, S], BF16, tag=f"kT{b}", name=f"kT{b}")

        def qk_chain(kind, w_all, dst):
            state = {}
            def emit_a():
                acc = ps_qkv.tile([128, RT], F32, tag="qkv",
                                  name=f"{kind}acc{rt}")
                for ec in range(4):
                    nc.tensor.matmul(acc[:], w_all[:, ec, :], xt[:, ec, :],
                                     start=(ec == 0), stop=False)
                state["acc"] = acc
            def emit_b():
                acc = state.pop("acc")
                for ec in range(4, NEC):
                    nc.tensor.matmul(acc[:], w_all[:, ec, :], xt[:, ec, :],
                                     start=False, stop=(ec == NEC - 1))
                raw = rawp.tile([128, RT], BF16, tag="raw",
                                name=f"{kind}raw{rt}")
                # DVE eviction: keep the Scalar engine free for exp, which
                # paces the attention phases these chains dribble into
                nc.vector.tensor_copy(raw[:], acc[:])
                rot = ps_qkv.tile([128, RT], F32, tag="qkv",
                                  name=f"{kind}rot{rt}")
                nc.tensor.matmul(rot[:], p2_sb[:], raw[:], start=True, stop=True)
                t1 = tmpp.tile([128, RT], F32, tag="ropet", name=f"{kind}t1_{rt}")
                nc.vector.tensor_mul(t1[:], raw[:], cos_sb[:, st:st + RT])
                t2 = tmpp.tile([128, RT], F32, tag="ropet", name=f"{kind}t2_{rt}")
                nc.vector.tensor_mul(t2[:], rot[:], sin_sb[:, st:st + RT])
                nc.vector.tensor_add(dst[:, st:st + RT], t1[:], t2[:])
            return [emit_a, emit_b]

        vstate = {}

        def v_head_chain(half):
            # v^T = wv.T @ x computed at full rate (N=512), half the e-chunks
            # per pop; the PE transpose in v_tail_chain flips it back to the
            # [k, hd] layout attn@v needs.
            def emit():
                if half == 0:
                    vacc = ps_qkv.tile([128, RT], F32, tag="qkv",
                                       name=f"vTacc{rt}")
                    vstate["ps"] = vacc
                vacc = vstate["ps"]
                for ec in range(4 * half, 4 * half + 4):
                    nc.tensor.matmul(vacc[:], wv_all[:, ec, :], xt[:, ec, :],
                                     start=(ec == 0), stop=(ec == NEC - 1))
                if half == 1:
                    vts = rawp.tile([128, RT], F32, tag="raw",
                                    name=f"vts{rt}")
                    nc.vector.tensor_copy(vts[:], vstate.pop("ps")[:])
                    vstate["sb"] = vts
            return emit

        def v_tail_chain(pair):
            def emit():
                vts = vstate["sb"]
                for sub in (2 * pair, 2 * pair + 1):
                    vtr = ps_qkv.tile([128, 128], F32, tag="qkv",
                                      name=f"vtr{rt}_{sub}")
                    nc.tensor.transpose(
                        vtr[:], vts[:, sub * 128:(sub + 1) * 128], id_sb[:])
                    kc = (rt % N_QT) * 4 + sub
                    for h in range(HPC):
                        vt = vp.tile([128, 65], BF16, tag=f"v{b}{h}{kc}",
                                     name=f"v{b}{h}{kc}")
                        nc.vector.tensor_copy(vt[:, 0:64],
                                              vtr[:, h * 64:(h + 1) * 64])
                        nc.vector.tensor_copy(vt[:, 64:65], ones_f32[:, 0:1])
                        v_sb[(b, h, kc)] = vt
            return emit

        return qk_chain("q", wq_all, qT_sb[b]) + \
               qk_chain("k", wk_all, kT_sb[b]) + \
               [v_head_chain(0), v_head_chain(1),
                v_tail_chain(0), v_tail_chain(1)]

    def proj_chains(b, hf):
        """Output projection for my 128 rows of (batch b, half hf).
        The recv load is emitted lazily by the first chain so that building
        the chain list never precedes the collective's emission."""
        state0 = {}
        def get_recv():
            if "t" not in state0:
                t = xtp.tile([128, NEC, 128], BF16, tag="recv", bufs=2,
                             name=f"recv{b}{hf}")
                nc.gpsimd.dma_start(
                    out=t[:], in_=recv_d[(b, hf)].rearrange("c p r -> p c r"))
                state0["t"] = t
            return state0["t"]
        chains = []
        for rblk in [hf]:
            for ft in range(2):
                state = {}
                def emit_a(rblk=rblk, ft=ft, state=state):
                    recv_sb = get_recv()
                    # qkv psum tag: free during attention (projection is done)
                    ops = ps_qkv.tile([128, 512], F32, tag="qkv",
                                      name=f"ops{b}_{rblk}_{ft}")
                    for ec in range(4):
                        nc.tensor.matmul(
                            ops[:],
                            recv_sb[:, ec, :],
                            w2_all[:, ec, ft * 512:(ft + 1) * 512],
                            start=(ec == 0), stop=False)
                    state["ops"] = ops
                def emit_b(rblk=rblk, ft=ft, state=state):
                    recv_sb = get_recv()
                    ops = state.pop("ops")
                    for ec in range(4, NEC):
                        nc.tensor.matmul(
                            ops[:],
                            recv_sb[:, ec, :],
                            w2_all[:, ec, ft * 512:(ft + 1) * 512],
                            start=False, stop=(ec == NEC - 1))
                    ot = tmpp.tile([128, 512], F32, tag="ropet",
                                   name=f"ot{b}_{rblk}_{ft}")
                    if b == 1 and hf == 1:
                        # kernel tail: exp stream is over, ACT is free
                        nc.scalar.copy(ot[:], ops[:])
                    else:
                        # runs during an attention stretch where exp keeps
                        # ACT busy: evict on DVE
                        nc.vector.tensor_copy(ot[:], ops[:])
                    # out rows: [b0h0, b0h1, b1h0, b1h1] blocks of 128
                    ob = 2 * b + rblk
                    nc.sync.dma_start(
                        out=out[ob * 128:(ob + 1) * 128,
                                ft * 512:(ft + 1) * 512],
                        in_=ot[:])
                chains.append(emit_a)
                chains.append(emit_b)
        return chains

    def emit_divide(b, qt, avs):
        """Divide by the softmax denominator (row 64 of av) and stage into
        the A2A send buffer. PE-free: broadcast via a DRAM bounce DMA.
        All DMAs on HWDGE queues (sync/scalar) so the collective sitting on
        the gpsimd queue can never block them."""
        last = (b == B - 1 and qt == N_QT - 1)
        for h in range(HPC):
            # evict the accumulator to SBUF immediately: releases the PSUM
            # slot so the next q-tile's attn@v never waits on this divide
            oraw = smallp.tile([65, QT], F32, tag="oraw", name=f"oraw{b}{h}{qt}")
            if last:
                nc.scalar.copy(oraw[:], avs[h][:])
            else:
                # exp paces the attention stream: keep evictions off ACT
                nc.vector.tensor_copy(oraw[:], avs[h][:])
            rcp = smallp.tile([1, QT], F32, tag="rcp", name=f"rcp{b}{h}{qt}")
            if qt == N_QT - 1:
                # batch tail: this divide gates the half's A2A while the
                # batch's exp stream is ending — compute 1/d = exp(-ln d) on
                # the emptying ACT pipe (~1us) instead of the 3.3us DVE
                # iterative reciprocal. Ln and Exp share one ACT table.
                lnd = smallp.tile([1, QT], F32, tag="lnd", name=f"lnd{b}{h}{qt}")
                nc.scalar.activation(lnd[:], oraw[64:65, :], LNF)
                nc.scalar.activation(rcp[:], lnd[:], EXPF, scale=-1.0)
            else:
                nc.vector.reciprocal(rcp[:], oraw[64:65, :])
            bcs = smallp.tile([64, QT], F32, tag="bcs", name=f"bcs{b}{h}{qt}")
            if last:
                # PE is idle at the kernel tail: broadcast via a K=1 matmul
                # instead of the DRAM-bounce DMA round trip
                bcq = ps_av.tile([64, QT], F32, tag="av", name=f"bcq{b}{h}{qt}")
                nc.tensor.matmul(bcq[:], ones_r[:], rcp[:], start=True,
                                 stop=True)
                nc.scalar.copy(bcs[:], bcq[:])
            else:
                rcp_d = dramp.tile([1, QT], F32, tag="rcpd", bufs=4,
                                   name=f"rcpd{b}{h}{qt}")
                nc.sync.dma_start(out=rcp_d[:], in_=rcp[:])
                bcast = bass.AP(tensor=rcp_d.tensor, offset=rcp_d.offset,
                                ap=[[0, 64]] + list(rcp_d.ap[1:]))
                nc.sync.dma_start(out=bcs[:], in_=bcast)
            odiv = smallp.tile([64, QT], BF16, tag="odiv", name=f"odiv{b}{h}{qt}")
            nc.vector.tensor_mul(odiv[:], oraw[0:64, :], bcs[:])
            # q-tile qt covers s in [512qt, 512qt+512): half hf = qt // 2,
            # destination cores j = 4*(qt%2) .. +4, 128 columns each
            hf = qt // 2
            for jj in range(4):
                j = 4 * (qt % 2) + jj
                nc.sync.dma_start(
                    out=send_d[(b, hf)][j, h * 64:(h + 1) * 64, :],
                    in_=odiv[:, jj * 128:(jj + 1) * 128])

    def emit_attention_batch(b, dribble):
        """All 4 q-tiles of a batch as one rolling pipeline over 64+LAG
        (qt, kc) units: scores+exp lead, attn@v trails by LAG units, the
        divide chain fires as each q-tile's accumulation completes.  One
        dribble chain (qkv projection / output projection) is popped every
        other unit to keep the in-order PE stream dense."""
        scale = 1.0 / math.sqrt(HD)
        NU = N_QT * N_KC
        LAG = 5
        pts = {}
        avs = {}
        for u in range(NU + LAG):
            if u < NU:
                qt, kc = divmod(u, N_KC)
                if kc == 0:
                    avs[qt] = [ps_av.tile([65, QT], F32, tag="av",
                                          name=f"av{b}{h}{qt}")
                               for h in range(HPC)]
                sps = ps_sps.tile([128, 2 * QT], F32, tag="sps",
                                  name=f"s{b}{qt}_{kc}")
                for h in range(HPC):
                    hof = h * 64
                    nc.tensor.matmul(
                        sps[:, h * QT:(h + 1) * QT],
                        kT_sb[b][hof:hof + 64, kc * KC:(kc + 1) * KC],
                        qT_sb[b][hof:hof + 64, qt * QT:(qt + 1) * QT],
                        start=True, stop=True)
                pt = pp.tile([128, 2 * QT], BF16, tag="p", name=f"p{b}{qt}_{kc}")
                nc.scalar.activation(pt[:], sps[:], EXPF, scale=scale)
                pts[u] = pt
            if u >= LAG:
                j = u - LAG
                qt2, kc2 = divmod(j, N_KC)
                for h in range(HPC):
                    nc.tensor.matmul(avs[qt2][h][:], v_sb[(b, h, kc2)][:],
                                     pts[j][:, h * QT:(h + 1) * QT],
                                     start=(kc2 == 0), stop=(kc2 == N_KC - 1))
                del pts[j]
                if kc2 == N_KC - 1:
                    emit_divide(b, qt2, avs.pop(qt2))
                    if qt2 == 1:
                        emit_a2a(b, 0)
            # one chain per two units, ramping up near the end so no
            # backlog remains to run as a monolithic lump afterwards
            if dribble and dribble[0][0] <= u and (
                    u % 2 == 1 or 2 * len(dribble) >= (NU + LAG - u)):
                dribble.pop(0)[1]()

    def emit_a2a(b, hf):
        nc.gpsimd.collective_compute(
            "AllToAll", mybir.AluOpType.bypass,
            replica_groups=[list(range(N_CORES))],
            ins=[send_d[(b, hf)].opt()], outs=[recv_d[(b, hf)].opt()])

    # ---------------- emission ----------------
    for rt in range(N_QT):             # batch-0 projection: pure PE stretch
        for chain in qkv_chains(rt):
            chain()
    # warm the collective path (cold-start ~8us); emitted here so the wait on
    # the gpsimd queue never delays the critical first x/weight loads
    cwu_s = dramp.tile([N_CORES, 8], F32, tag="cwus", name="cwu_s")
    cwu_r = dramp.tile([N_CORES, 8], F32, tag="cwur", name="cwu_r")
    nc.sync.dma_start(out=cwu_s.rearrange("c r -> (c r)")[None, :],
                      in_=ones_f32[0:1, 0:64])
    nc.gpsimd.collective_compute(
        "AllToAll", mybir.AluOpType.bypass,
        replica_groups=[list(range(N_CORES))],
        ins=[cwu_s.opt()], outs=[cwu_r.opt()])

    # batch-0 attention with batch-1 qkv dribbled in, then batch-0 half-0
    # projections once A2A(0,0) has landed (~mid-batch)
    dribble = [(1, c) for rt in range(N_QT, 2 * N_QT) for c in qkv_chains(rt)]
    dribble += [(56, c) for c in proj_chains(0, 0)]
    emit_attention_batch(0, dribble)
    for _, chain in dribble:
        chain()
    del dribble[:]
    emit_a2a(0, 1)                     # second half, fires at batch-0 end

    # batch-1 attention: batch-0 half-1 projection early (its A2A fires at
    # batch-0 end), batch-1 half 0's A2A fires mid-batch, its projection at
    # the tail
    dribble = [(4, c) for c in proj_chains(0, 1)]
    dribble += [(48, c) for c in proj_chains(1, 0)]
    emit_attention_batch(1, dribble)
    for _, chain in dribble:
        chain()
    emit_a2a(1, 1)
    for chain in proj_chains(1, 1):
        chain()
    ctx.close()


def _host_prep(x, w1, w2):
    import ml_dtypes
    bf16 = ml_dtypes.bfloat16
    x = np.asarray(x, dtype=np.float32)
    w1 = np.asarray(w1, dtype=np.float32)
    w2 = np.asarray(w2, dtype=np.float32)

    xT = np.ascontiguousarray(x.reshape(R, E).T.astype(bf16))      # [E, R]
    w2T = np.ascontiguousarray(w2.T.astype(bf16))                  # [E, E]

    theta = 1.0 / (BASE ** (np.arange(0, HD, 2, dtype=np.float32) / HD))
    enc = np.arange(S, dtype=np.float32)[:, None] * theta[None, :]
    enc = np.repeat(enc, 2, axis=-1)                      # [s, 64]
    cos1 = np.cos(enc).T.astype(np.float32)               # [64, S]
    sin1 = np.sin(enc).T.astype(np.float32)
    cosT = np.ascontiguousarray(np.concatenate([cos1, cos1], axis=0))
    sinT = np.ascontiguousarray(np.concatenate([sin1, sin1], axis=0))

    m64 = np.zeros((HD, HD), dtype=np.float32)
    for i in range(HD // 2):
        m64[2 * i, 2 * i + 1] = -1.0
        m64[2 * i + 1, 2 * i] = 1.0
    m128 = np.zeros((128, 128), dtype=np.float32)
    m128[:64, :64] = m64
    m128[64:, 64:] = m64
    p2T = np.ascontiguousarray(m128.T.astype(bf16))

    in_maps = []
    for c in range(N_CORES):
        hA, hB = HPC * c, HPC * c + 1
        def rows(base):
            return np.concatenate(
                [w1[base + hA * HD: base + (hA + 1) * HD, :],
                 w1[base + hB * HD: base + (hB + 1) * HD, :]], axis=0)
        in_maps.append({
            "xT": xT,
            "wqT": np.ascontiguousarray(rows(0).T.astype(bf16)),
            "wkT": np.ascontiguousarray(rows(E).T.astype(bf16)),
            "wvT": np.ascontiguousarray(rows(2 * E).T.astype(bf16)),
            "w2T": w2T,
            "cosT": cosT,
            "sinT": sinT,
            "p2T": p2T,
        })
    return in_maps


def kernel(x, w1, w2, _trace=False):
    if "nc" not in _COMPILED:
        _COMPILED["nc"] = _build_nc()
    nc = _COMPILED["nc"]
    in_maps = _host_prep(x, w1, w2)
    res = run_bass_kernel_spmd(nc, in_maps, core_ids=list(range(N_CORES)),
                               trace=_trace)
    _COMPILED["last_result"] = res
    # core c returns [512, E] as four 128-row blocks:
    # [b0 s=128c.., b0 s=1024+128c.., b1 s=128c.., b1 s=1024+128c..]
    full = np.empty((B, S, E), dtype=np.float32)
    for c in range(N_CORES):
        blk = res.results[c]["out"]
        full[0, 128 * c:128 * (c + 1)] = blk[0:128]
        full[0, 1024 + 128 * c:1024 + 128 * (c + 1)] = blk[128:256]
        full[1, 128 * c:128 * (c + 1)] = blk[256:384]
        full[1, 1024 + 128 * c:1024 + 128 * (c + 1)] = blk[384:512]
    return full


# revision 15
# speedup vs baseline: 1.0496x; 1.0496x over previous
"""Trainium2 Bass kernel for nn_Attention_12000138625343.

Full multi-head attention layer (B=2, S=2048, E=1024, H=16, hd=64, interleaved
RoPE on q/k, non-causal softmax) run tensor-parallel over 8 NeuronCores:

  - heads sharded 2-per-core (w1 columns / qkv projection sharded),
  - x replicated, passed pre-transposed [E, B*S] in bf16 so the contraction
    dim lands on SBUF partitions,
  - all matmul operands in bf16 (PSUM accumulation stays fp32): halves HBM
    traffic, SBUF footprint, and the A2A payload,
  - bulk loads ride the gpsimd SWDGE queue (descriptor-gen ~1us each, the
    transfers fan out across the DMA-engine pool); HWDGE queues serialize
    bulk data so they only carry small sends,
  - scores computed transposed [k, q]; the two heads' K=64 score matmuls are
    packed into disjoint PE row-groups, one exp instruction covers both
    heads' [128, 1024] PSUM block,
  - attn@v accumulates rolling per k-chunk with a ones-column appended to v
    producing the softmax denominator in row 64,
  - the softmax divide happens AFTER the A2A on the receive side: each core
    sends raw o^T plus the denominator row per head; the receiver
    reciprocates all 16 denominators in one partition-parallel DVE op,
    broadcasts via a DRAM-bounce DMA, and multiplies once per half. This
    keeps the entire divide chain off the kernel tail,
  - AllToAll halves are "diagonal": half A = {qt0, qt3}, half B = {qt1, qt2}
    (each covers all 8 destination cores). Half B completes at 3/4 of the
    batch so its collective and projection overlap the remaining attention
    units; only half A's 0.26 MB collective remains at the batch end. Cores
    0-3 receive row-block 0 from half A, cores 4-7 row-block 1 (host gather
    compensates),
  - batch-1 qkv projection / output projection matmul chains are dribbled
    into the attention k-chunk loop so the in-order PE stream never starves
    the exp pipeline for long,
  - each core owns 2 x 128 rows of each batch; host reassembles.
"""

import math

import numpy as np

import concourse.bass as bass
import concourse.mybir as mybir
import concourse.tile as tile
from concourse import bacc
from concourse.bass_utils import run_bass_kernel_spmd
from concourse.masks import make_identity

B, S, E, H = 2, 2048, 1024, 16
HD = E // H  # 64
BASE = 10000.0
N_CORES = 8
HPC = H // N_CORES       # heads per core = 2
R = B * S                # 4096 flattened rows
RT = 512                 # rows per r-tile
NEC = E // 128           # 8 e-chunks of 128
QT = 512                 # q columns per q-tile
N_QT = S // QT           # 4 q-tiles per batch
KC = 128                 # k rows per k-chunk
N_KC = S // KC           # 16 k-chunks per batch
RPB = S // N_CORES       # rows per core per batch = 256

F32 = mybir.dt.float32
BF16 = mybir.dt.bfloat16
EXPF = mybir.ActivationFunctionType.Exp

# diagonal A2A halves: each covers all 8 destination cores
HALF = {0: "A", 1: "B", 2: "B", 3: "A"}

_COMPILED = {}


def _build_nc():
    nc = bacc.Bacc("TRN2", target_bir_lowering=False, debug=False,
                   num_devices=N_CORES)

    xT = nc.dram_tensor("xT", [E, R], BF16, kind="ExternalInput").ap()
    wqT = nc.dram_tensor("wqT", [E, 128], BF16, kind="ExternalInput").ap()
    wkT = nc.dram_tensor("wkT", [E, 128], BF16, kind="ExternalInput").ap()
    wvT = nc.dram_tensor("wvT", [E, 128], BF16, kind="ExternalInput").ap()
    w2T = nc.dram_tensor("w2T", [E, E], BF16, kind="ExternalInput").ap()
    cosT = nc.dram_tensor("cosT", [128, S], F32, kind="ExternalInput").ap()
    sinT = nc.dram_tensor("sinT", [128, S], F32, kind="ExternalInput").ap()
    p2T = nc.dram_tensor("p2T", [128, 128], BF16, kind="ExternalInput").ap()
    out = nc.dram_tensor("out", [2 * RPB, E], F32, kind="ExternalOutput").ap()

    with tile.TileContext(nc) as tc:
        _emit(tc, nc, xT, wqT, wkT, wvT, w2T, cosT, sinT, p2T, out)
    nc.compile()
    return nc


def _emit(tc, nc, xT, wqT, wkT, wvT, w2T, cosT, sinT, p2T, out):
    import contextlib
    ctx = contextlib.ExitStack()
    consts = ctx.enter_context(tc.tile_pool(name="consts", bufs=1))
    xtp = ctx.enter_context(tc.tile_pool(name="xtp", bufs=2))
    qkp = ctx.enter_context(tc.tile_pool(name="qkp", bufs=1))
    rawp = ctx.enter_context(tc.tile_pool(name="rawp", bufs=2))
    tmpp = ctx.enter_context(tc.tile_pool(name="tmpp", bufs=2))
    vp = ctx.enter_context(tc.tile_pool(name="vp", bufs=1))
    pp = ctx.enter_context(tc.tile_pool(name="pp", bufs=7))
    smallp = ctx.enter_context(tc.tile_pool(name="smallp", bufs=2))
    dramp = ctx.enter_context(tc.tile_pool(name="dramp", bufs=1, space="DRAM"))
    # PSUM budget (8 banks): qkv-shared 2 + sps 2 x 2 + av 2 = 8
    ps_qkv = ctx.enter_context(tc.tile_pool(name="ps_qkv", bufs=2, space="PSUM"))
    ps_sps = ctx.enter_context(tc.tile_pool(name="ps_sps", bufs=2, space="PSUM"))
    ps_av = ctx.enter_context(tc.tile_pool(name="ps_av", bufs=2, space="PSUM"))

    # ---- bulk input loads on the gpsimd SWDGE queue (parallel transfers);
    # with xt bufs=8 nothing waits in-FIFO, so the collective triggers
    # emitted later are never delayed ----
    wq_all = consts.tile([128, NEC, 128], BF16, tag="wq", name="wq_all")
    nc.sync.dma_start(out=wq_all[:], in_=wqT.rearrange("(c p) f -> p c f", p=128))

    xts = []
    for rt in range(2 * N_QT):
        t = xtp.tile([128, NEC, RT], BF16, tag="xt", bufs=8, name=f"xt_{rt}")
        nc.gpsimd.dma_start(
            out=t[:],
            in_=xT.rearrange("(c p) r -> p c r", p=128)[:, :, rt * RT:(rt + 1) * RT])
        xts.append(t)
        if rt == 0:
            wk_all = consts.tile([128, NEC, 128], BF16, tag="wk", name="wk_all")
            nc.gpsimd.dma_start(
                out=wk_all[:], in_=wkT.rearrange("(c p) f -> p c f", p=128))
            wv_all = consts.tile([128, NEC, 128], BF16, tag="wv", name="wv_all")
            nc.gpsimd.dma_start(
                out=wv_all[:], in_=wvT.rearrange("(c p) f -> p c f", p=128))
            p2_sb = consts.tile([128, 128], BF16, tag="p2", name="p2_sb")
            nc.gpsimd.dma_start(out=p2_sb[:], in_=p2T[:, :])
        if rt == 1:
            cos_sb = consts.tile([128, S], F32, tag="cos", name="cos_sb")
            nc.gpsimd.dma_start(out=cos_sb[:], in_=cosT[:, :])
            sin_sb = consts.tile([128, S], F32, tag="sin", name="sin_sb")
            nc.gpsimd.dma_start(out=sin_sb[:], in_=sinT[:, :])
    # w2: 2 MB, overlaps the batch-0 qkv/attention stretch
    w2_all = consts.tile([128, NEC, E], BF16, tag="w2", name="w2_all")
    nc.gpsimd.dma_start(out=w2_all[:], in_=w2T.rearrange("(c p) f -> p c f", p=128))

    ones_f32 = consts.tile([128, 64], F32, tag="ones32", name="ones_f32")
    nc.vector.memset(ones_f32[:], 1.0)
    id_sb = consts.tile([128, 128], F32, tag="idm", name="id_sb")
    make_identity(nc, id_sb[:])

    # A2A buffers, one pair per (batch, half): dest core j's chunk is
    # [2 heads, 65 rows (64 o^T + denominator), 128 s-cols]
    send_d = {(b, hf): dramp.tile([N_CORES, HPC, 65, 128], BF16,
                                  name=f"send{b}{hf}")
              for b in range(B) for hf in ("A", "B")}
    recv_d = {(b, hf): dramp.tile([N_CORES, HPC, 65, 128], BF16,
                                  name=f"recv{b}{hf}")
              for b in range(B) for hf in ("A", "B")}

    qT_sb, kT_sb, v_sb = {}, {}, {}

    def qkv_chains(rt):
        """Return a list of closures, each emitting one matmul chain (+ its
        epilogue) for r-tile rt. Callers dribble these between attention
        steps to keep the in-order PE stream dense but never monolithic."""
        b, st = rt // N_QT, (rt % N_QT) * RT
        xt = xts[rt]

        if b not in qT_sb:
            qT_sb[b] = qkp.tile([128, S], BF16, tag=f"qT{b}", name=f"qT{b}")
            kT_sb[b] = qkp.tile([128, S], BF16, tag=f"kT{b}", name=f"kT{b}")

        def qk_chain(kind, w_all, dst):
            state = {}
            def emit_a():
                acc = ps_qkv.tile([128, RT], F32, tag="qkv",
                                  name=f"{kind}acc{rt}")
                for ec in range(4):
                    nc.tensor.matmul(acc[:], w_all[:, ec, :], xt[:, ec, :],
                                     start=(ec == 0), stop=False)
                state["acc"] = acc
            def emit_b():
                acc = state.pop("acc")
                for ec in range(4, NEC):
                    nc.tensor.matmul(acc[:], w_all[:, ec, :], xt[:, ec, :],
                                     start=False, stop=(ec == NEC - 1))
                raw = rawp.tile([128, RT], BF16, tag="raw",
                                name=f"{kind}raw{rt}")
                # DVE eviction: keep the Scalar engine free for exp, which
                # paces the attention phases these chains dribble into
                nc.vector.tensor_copy(raw[:], acc[:])
                rot = ps_qkv.tile([128, RT], F32, tag="qkv",
                                  name=f"{kind}rot{rt}")
                nc.tensor.matmul(rot[:], p2_sb[:], raw[:], start=True, stop=True)
                t1 = tmpp.tile([128, RT], F32, tag="ropet", name=f"{kind}t1_{rt}")
                nc.vector.tensor_mul(t1[:], raw[:], cos_sb[:, st:st + RT])
                t2 = tmpp.tile([128, RT], F32, tag="ropet", name=f"{kind}t2_{rt}")
                nc.vector.tensor_mul(t2[:], rot[:], sin_sb[:, st:st + RT])
                nc.vector.tensor_add(dst[:, st:st + RT], t1[:], t2[:])
            return [emit_a, emit_b]

        vstate = {}

        def v_head_chain(half):
            # v^T = wv.T @ x computed at full rate (N=512), half the e-chunks
            # per pop; the PE transpose in v_tail_chain flips it back to the
            # [k, hd] layout attn@v needs.
            def emit():
                if half == 0:
                    vacc = ps_qkv.tile([128, RT], F32, tag="qkv",
                                       name=f"vTacc{rt}")
                    vstate["ps"] = vacc
                vacc = vstate["ps"]
                for ec in range(4 * half, 4 * half + 4):
                    nc.tensor.matmul(vacc[:], wv_all[:, ec, :], xt[:, ec, :],
                                     start=(ec == 0), stop=(ec == NEC - 1))
                if half == 1:
                    vts = rawp.tile([128, RT], F32, tag="raw",
                                    name=f"vts{rt}")
                    nc.vector.tensor_copy(vts[:], vstate.pop("ps")[:])
                    vstate["sb"] = vts
            return emit

        def v_tail_chain(pair):
            def emit():
                vts = vstate["sb"]
                for sub in (2 * pair, 2 * pair + 1):
                    vtr = ps_qkv.tile([128, 128], F32, tag="qkv",
                                      name=f"vtr{rt}_{sub}")
                    nc.tensor.transpose(
                        vtr[:], vts[:, sub * 128:(sub + 1) * 128], id_sb[:])
                    kc = (rt % N_QT) * 4 + sub
                    for h in range(HPC):
                        vt = vp.tile([128, 65], BF16, tag=f"v{b}{h}{kc}",
                                     name=f"v{b}{h}{kc}")
                        nc.vector.tensor_copy(vt[:, 0:64],
                                              vtr[:, h * 64:(h + 1) * 64])
                        nc.vector.tensor_copy(vt[:, 64:65], ones_f32[:, 0:1])
                        v_sb[(b, h, kc)] = vt
            return emit

        return qk_chain("q", wq_all, qT_sb[b]) + \
               qk_chain("k", wk_all, kT_sb[b]) + \
               [v_head_chain(0), v_head_chain(1),
                v_tail_chain(0), v_tail_chain(1)]

    def proj_chains(b, hf):
        """Output projection for my 128 rows of (batch b, diagonal half hf).
        The recv load + softmax divide are emitted lazily by the first chain
        so they never precede the collective's emission."""
        state0 = {}
        def get_odv():
            if "odv" not in state0:
                # o^T rows: e-row within source chunk c is h*64+p
                recv_sb = xtp.tile([128, NEC, 128], BF16, tag="recv", bufs=2,
                                   name=f"recv{b}{hf}")
                for h in range(HPC):
                    nc.gpsimd.dma_start(
                        out=recv_sb[h * 64:(h + 1) * 64, :, :],
                        in_=recv_d[(b, hf)][:, h, 0:64, :].rearrange(
                            "c p r -> p c r"))
                # denominator rows, one partition per (source, head)
                dn = smallp.tile([16, 128], BF16, tag="dn", name=f"dn{b}{hf}")
                nc.gpsimd.dma_start(
                    out=dn[:],
                    in_=recv_d[(b, hf)][:, :, 64:65, :].rearrange(
                        "c h p r -> (c h p) r"))
                # all 16 reciprocals in one partition-parallel op (~0.9us)
                rcp16 = smallp.tile([16, 128], F32, tag="rcp16",
                                    name=f"rcp16{b}{hf}")
                nc.vector.reciprocal(rcp16[:], dn[:])
                # broadcast [16,128] -> [128, c, 128] via a DRAM bounce:
                # partition (h*64+rep) reads rcp row c*2+h
                rcp_dr = dramp.tile([16, 128], F32, tag="rcpd", bufs=2,
                                    name=f"rcpd{b}{hf}")
                nc.sync.dma_start(out=rcp_dr[:], in_=rcp16[:])
                bcast = smallp.tile([128, NEC, 128], F32, tag="bcast",
                                    name=f"bcast{b}{hf}")
                for h in range(HPC):
                    bc_ap = bass.AP(
                        tensor=rcp_dr.tensor,
                        offset=rcp_dr.offset + h * 128,
                        ap=[[0, 64], [HPC * 128, N_CORES], [1, 128]])
                    nc.sync.dma_start(out=bcast[h * 64:(h + 1) * 64, :, :],
                                      in_=bc_ap)
                odv = xtp.tile([128, NEC, 128], BF16, tag="odv", bufs=2,
                               name=f"odv{b}{hf}")
                nc.vector.tensor_mul(odv[:], recv_sb[:], bcast[:])
                state0["odv"] = odv
            return state0["odv"]
        chains = []
        rblk = {"A": 0, "B": 1}[hf]     # for cores 0-3; host swaps for 4-7
        for ft in range(2):
            state = {}
            def emit_a(ft=ft, state=state):
                odv = get_odv()
                # qkv psum tag: free during attention (projection is done)
                ops = ps_qkv.tile([128, 512], F32, tag="qkv",
                                  name=f"ops{b}_{rblk}_{ft}")
                for ec in range(4):
                    nc.tensor.matmul(
                        ops[:],
                        odv[:, ec, :],
                        w2_all[:, ec, ft * 512:(ft + 1) * 512],
                        start=(ec == 0), stop=False)
                state["ops"] = ops
            def emit_b(ft=ft, state=state):
                odv = get_odv()
                ops = state.pop("ops")
                for ec in range(4, NEC):
                    nc.tensor.matmul(
                        ops[:],
                        odv[:, ec, :],
                        w2_all[:, ec, ft * 512:(ft + 1) * 512],
                        start=False, stop=(ec == NEC - 1))
                ot = tmpp.tile([128, 512], F32, tag="ropet",
                               name=f"ot{b}_{rblk}_{ft}")
                if b == 1 and hf == "A":
                    # kernel tail: exp stream is over, ACT is free
                    nc.scalar.copy(ot[:], ops[:])
                else:
                    # runs during an attention stretch where exp keeps ACT
                    # busy: evict on DVE
                    nc.vector.tensor_copy(ot[:], ops[:])
                ob = 2 * b + rblk
                nc.sync.dma_start(
                    out=out[ob * 128:(ob + 1) * 128,
                            ft * 512:(ft + 1) * 512],
                    in_=ot[:])
            chains.append(emit_a)
            chains.append(emit_b)
        return chains

    def emit_stage(b, qt, avs):
        """Evict the attn@v accumulator (o^T raw + denominator row) straight
        into the A2A send buffer; the divide happens on the receive side."""
        last = (b == B - 1 and qt == N_QT - 1)
        hf = HALF[qt]
        jbase = 4 * (qt % 2)
        for h in range(HPC):
            # evict immediately: releases the PSUM slot so the next q-tile's
            # attn@v never waits
            oraw = smallp.tile([65, QT], BF16, tag="oraw",
                               name=f"oraw{b}{h}{qt}")
            if last:
                nc.scalar.copy(oraw[:], avs[h][:])
            else:
                # exp paces the attention stream: keep evictions off ACT
                nc.vector.tensor_copy(oraw[:], avs[h][:])
            for jj in range(4):
                nc.sync.dma_start(
                    out=send_d[(b, hf)][jbase + jj, h, :, :],
                    in_=oraw[:, jj * 128:(jj + 1) * 128])

    def emit_attention_batch(b, dribble):
        """All 4 q-tiles of a batch as one rolling pipeline over 64+LAG
        (qt, kc) units: scores+exp lead, attn@v trails by LAG units, the
        staging fires as each q-tile's accumulation completes. One dribble
        chain (qkv projection / output projection) is popped every other
        unit to keep the in-order PE stream dense."""
        scale = 1.0 / math.sqrt(HD)
        NU = N_QT * N_KC
        LAG = 5
        pts = {}
        avs = {}
        for u in range(NU + LAG):
            if u < NU:
                qt, kc = divmod(u, N_KC)
                if kc == 0:
                    avs[qt] = [ps_av.tile([65, QT], F32, tag="av",
                                          name=f"av{b}{h}{qt}")
                               for h in range(HPC)]
                sps = ps_sps.tile([128, 2 * QT], F32, tag="sps",
                                  name=f"s{b}{qt}_{kc}")
                for h in range(HPC):
                    hof = h * 64
                    nc.tensor.matmul(
                        sps[:, h * QT:(h + 1) * QT],
                        kT_sb[b][hof:hof + 64, kc * KC:(kc + 1) * KC],
                        qT_sb[b][hof:hof + 64, qt * QT:(qt + 1) * QT],
                        start=True, stop=True)
                pt = pp.tile([128, 2 * QT], BF16, tag="p", name=f"p{b}{qt}_{kc}")
                nc.scalar.activation(pt[:], sps[:], EXPF, scale=scale)
                pts[u] = pt
            if u >= LAG:
                j = u - LAG
                qt2, kc2 = divmod(j, N_KC)
                for h in range(HPC):
                    nc.tensor.matmul(avs[qt2][h][:], v_sb[(b, h, kc2)][:],
                                     pts[j][:, h * QT:(h + 1) * QT],
                                     start=(kc2 == 0), stop=(kc2 == N_KC - 1))
                del pts[j]
                if kc2 == N_KC - 1:
                    emit_stage(b, qt2, avs.pop(qt2))
                    if qt2 == 2:
                        emit_a2a(b, "B")
            # one chain per two units, ramping up near the end so no
            # backlog remains to run as a monolithic lump afterwards
            if dribble and dribble[0][0] <= u and (
                    u % 2 == 1 or 2 * len(dribble) >= (NU + LAG - u)):
                dribble.pop(0)[1]()

    def emit_a2a(b, hf):
        nc.gpsimd.collective_compute(
            "AllToAll", mybir.AluOpType.bypass,
            replica_groups=[list(range(N_CORES))],
            ins=[send_d[(b, hf)].opt()], outs=[recv_d[(b, hf)].opt()])

    # ---------------- emission ----------------
    for rt in range(N_QT):             # batch-0 projection: pure PE stretch
        for chain in qkv_chains(rt):
            chain()
    # warm the collective path (cold-start ~8us); emitted here so the wait on
    # the gpsimd queue never delays the critical first x/weight loads
    cwu_s = dramp.tile([N_CORES, 8], F32, tag="cwus", name="cwu_s")
    cwu_r = dramp.tile([N_CORES, 8], F32, tag="cwur", name="cwu_r")
    nc.sync.dma_start(out=cwu_s.rearrange("c r -> (c r)")[None, :],
                      in_=ones_f32[0:1, 0:64])
    nc.gpsimd.collective_compute(
        "AllToAll", mybir.AluOpType.bypass,
        replica_groups=[list(range(N_CORES))],
        ins=[cwu_s.opt()], outs=[cwu_r.opt()])

    # batch-0 attention with batch-1 qkv dribbled in; A2A(0,B) fires at 3/4
    dribble = [(1, c) for rt in range(N_QT, 2 * N_QT) for c in qkv_chains(rt)]
    emit_attention_batch(0, dribble)
    for _, chain in dribble:
        chain()
    del dribble[:]
    emit_a2a(0, "A")                   # fires at batch-0 end

    # batch-1 attention: batch-0 projections early (both its A2As have
    # landed), batch-1 half B's A2A fires at 3/4, its projection at the tail
    dribble = [(2, c) for c in proj_chains(0, "B")]
    dribble += [(14, c) for c in proj_chains(0, "A")]
    dribble += [(62, c) for c in proj_chains(1, "B")]
    emit_attention_batch(1, dribble)
    for _, chain in dribble:
        chain()
    emit_a2a(1, "A")
    for chain in proj_chains(1, "A"):
        chain()
    ctx.close()


def _host_prep(x, w1, w2):
    import ml_dtypes
    bf16 = ml_dtypes.bfloat16
    x = np.asarray(x, dtype=np.float32)
    w1 = np.asarray(w1, dtype=np.float32)
    w2 = np.asarray(w2, dtype=np.float32)

    xT = np.ascontiguousarray(x.reshape(R, E).T.astype(bf16))      # [E, R]
    w2T = np.ascontiguousarray(w2.T.astype(bf16))                  # [E, E]

    theta = 1.0 / (BASE ** (np.arange(0, HD, 2, dtype=np.float32) / HD))
    enc = np.arange(S, dtype=np.float32)[:, None] * theta[None, :]
    enc = np.repeat(enc, 2, axis=-1)                      # [s, 64]
    cos1 = np.cos(enc).T.astype(np.float32)               # [64, S]
    sin1 = np.sin(enc).T.astype(np.float32)
    cosT = np.ascontiguousarray(np.concatenate([cos1, cos1], axis=0))
    sinT = np.ascontiguousarray(np.concatenate([sin1, sin1], axis=0))

    m64 = np.zeros((HD, HD), dtype=np.float32)
    for i in range(HD // 2):
        m64[2 * i, 2 * i + 1] = -1.0
        m64[2 * i + 1, 2 * i] = 1.0
    m128 = np.zeros((128, 128), dtype=np.float32)
    m128[:64, :64] = m64
    m128[64:, 64:] = m64
    p2T = np.ascontiguousarray(m128.T.astype(bf16))

    in_maps = []
    for c in range(N_CORES):
        hA, hB = HPC * c, HPC * c + 1
        def rows(base):
            return np.concatenate(
                [w1[base + hA * HD: base + (hA + 1) * HD, :],
                 w1[base + hB * HD: base + (hB + 1) * HD, :]], axis=0)
        in_maps.append({
            "xT": xT,
            "wqT": np.ascontiguousarray(rows(0).T.astype(bf16)),
            "wkT": np.ascontiguousarray(rows(E).T.astype(bf16)),
            "wvT": np.ascontiguousarray(rows(2 * E).T.astype(bf16)),
            "w2T": w2T,
            "cosT": cosT,
            "sinT": sinT,
            "p2T": p2T,
        })
    return in_maps


def kernel(x, w1, w2, _trace=False):
    if "nc" not in _COMPILED:
        _COMPILED["nc"] = _build_nc()
    nc = _COMPILED["nc"]
    in_maps = _host_prep(x, w1, w2)
    res = run_bass_kernel_spmd(nc, in_maps, core_ids=list(range(N_CORES)),
                               trace=_trace)
    _COMPILED["last_result"] = res
    # core c returns [512, E] as four 128-row blocks written per (batch,
    # half): [b0 A, b0 B, b1 A, b1 B]. Half A carries s-rows 128c for cores
    # 0-3 but 1024+128c for cores 4-7 (diagonal halves); B is the opposite.
    full = np.empty((B, S, E), dtype=np.float32)
    for c in range(N_CORES):
        blk = res.results[c]["out"]
        lo, hi = (0, 1) if c < 4 else (1, 0)   # blk index carrying s=128c
        for b in range(B):
            full[b, 128 * c:128 * (c + 1)] = blk[(2 * b + lo) * 128:
                                                 (2 * b + lo + 1) * 128]
            full[b, 1024 + 128 * c:1024 + 128 * (c + 1)] = \
                blk[(2 * b + hi) * 128:(2 * b + hi + 1) * 128]
    return full


# revision 29
# speedup vs baseline: 1.1299x; 1.0765x over previous
"""Trainium2 Bass kernel for nn_Attention_12000138625343.

Full multi-head attention layer (B=2, S=2048, E=1024, H=16, hd=64, interleaved
RoPE on q/k, non-causal softmax) run tensor-parallel over 8 NeuronCores:

  - heads sharded 2-per-core (w1 columns / qkv projection sharded),
  - x replicated, passed pre-transposed [E, B*S] in bf16 so the contraction
    dim lands on SBUF partitions,
  - all matmul operands in bf16 (PSUM accumulation stays fp32): halves HBM
    traffic, SBUF footprint, and the A2A payload,
  - bulk loads ride the gpsimd SWDGE queue (descriptor-gen ~1us each, the
    transfers fan out across the DMA-engine pool); HWDGE queues serialize
    bulk data so they only carry small sends,
  - scores computed transposed [k, q]; the two heads' K=64 score matmuls are
    packed into disjoint PE row-groups, one exp instruction covers both
    heads' [128, 1024] PSUM block,
  - attn@v accumulates rolling per k-chunk with a ones-column appended to v
    producing the softmax denominator in row 64,
  - the softmax divide happens AFTER the A2A on the receive side: each core
    sends raw o^T plus the denominator row per head; the receiver
    reciprocates all 16 denominators in one partition-parallel DVE op,
    broadcasts via a DRAM-bounce DMA, and multiplies once per half. This
    keeps the entire divide chain off the kernel tail,
  - AllToAll halves are "diagonal": half A = {qt0, qt3}, half B = {qt1, qt2}
    (each covers all 8 destination cores). Half B completes at 3/4 of the
    batch so its collective and projection overlap the remaining attention
    units; only half A's 0.26 MB collective remains at the batch end. Cores
    0-3 receive row-block 0 from half A, cores 4-7 row-block 1 (host gather
    compensates),
  - batch-1 qkv projection / output projection matmul chains are dribbled
    into the attention k-chunk loop so the in-order PE stream never starves
    the exp pipeline for long,
  - each core owns 2 x 128 rows of each batch; host reassembles.
"""

import math

import numpy as np

import concourse.bass as bass
import concourse.mybir as mybir
import concourse.tile as tile
from concourse import bacc
from concourse.bass_utils import run_bass_kernel_spmd
from concourse.masks import make_identity

B, S, E, H = 2, 2048, 1024, 16
HD = E // H  # 64
BASE = 10000.0
N_CORES = 8
HPC = H // N_CORES       # heads per core = 2
R = B * S                # 4096 flattened rows
RT = 512                 # rows per r-tile
NEC = E // 128           # 8 e-chunks of 128
QT = 512                 # q columns per q-tile
N_QT = S // QT           # 4 q-tiles per batch
KC = 128                 # k rows per k-chunk
N_KC = S // KC           # 16 k-chunks per batch
RPB = S // N_CORES       # rows per core per batch = 256

F32 = mybir.dt.float32
BF16 = mybir.dt.bfloat16
FP8 = mybir.dt.float8e4
EXPF = mybir.ActivationFunctionType.Exp
DROW = mybir.MatmulPerfMode.DoubleRow
# softmax numerator/denominator are both scaled by 1/EXPC (ratio unchanged);
# keeps exp(s)/EXPC <= 240 (TRN fp8e4 max) for any realistic score
EXPC = 4.0

# diagonal A2A halves: each covers all 8 destination cores
HALF = {0: "A", 1: "B", 2: "B", 3: "A"}

_COMPILED = {}


def _build_nc():
    nc = bacc.Bacc("TRN2", target_bir_lowering=False, debug=False,
                   num_devices=N_CORES)

    xT = nc.dram_tensor("xT", [E, R], BF16, kind="ExternalInput").ap()
    wqT = nc.dram_tensor("wqT", [E, 128], BF16, kind="ExternalInput").ap()
    wkT = nc.dram_tensor("wkT", [E, 128], BF16, kind="ExternalInput").ap()
    wvT = nc.dram_tensor("wvT", [E, 128], BF16, kind="ExternalInput").ap()
    w2T = nc.dram_tensor("w2T", [E, E], BF16, kind="ExternalInput").ap()
    cosT = nc.dram_tensor("cosT", [128, S], F32, kind="ExternalInput").ap()
    sinT = nc.dram_tensor("sinT", [128, S], F32, kind="ExternalInput").ap()
    p2T = nc.dram_tensor("p2T", [128, 128], BF16, kind="ExternalInput").ap()
    out = nc.dram_tensor("out", [2 * RPB, E], F32, kind="ExternalOutput").ap()

    with tile.TileContext(nc) as tc:
        _emit(tc, nc, xT, wqT, wkT, wvT, w2T, cosT, sinT, p2T, out)
    nc.compile()
    return nc


def _emit(tc, nc, xT, wqT, wkT, wvT, w2T, cosT, sinT, p2T, out):
    import contextlib
    ctx = contextlib.ExitStack()
    consts = ctx.enter_context(tc.tile_pool(name="consts", bufs=1))
    xtp = ctx.enter_context(tc.tile_pool(name="xtp", bufs=2))
    qkp = ctx.enter_context(tc.tile_pool(name="qkp", bufs=1))
    rawp = ctx.enter_context(tc.tile_pool(name="rawp", bufs=2))
    tmpp = ctx.enter_context(tc.tile_pool(name="tmpp", bufs=2))
    vp = ctx.enter_context(tc.tile_pool(name="vp", bufs=1))
    pp = ctx.enter_context(tc.tile_pool(name="pp", bufs=7))
    smallp = ctx.enter_context(tc.tile_pool(name="smallp", bufs=2))
    dramp = ctx.enter_context(tc.tile_pool(name="dramp", bufs=1, space="DRAM"))
    # PSUM budget (8 banks): qkv-shared 2 + sps 2 x 2 + av 2 = 8
    ps_qkv = ctx.enter_context(tc.tile_pool(name="ps_qkv", bufs=2, space="PSUM"))
    ps_sps = ctx.enter_context(tc.tile_pool(name="ps_sps", bufs=2, space="PSUM"))
    ps_av = ctx.enter_context(tc.tile_pool(name="ps_av", bufs=2, space="PSUM"))

    # ---- bulk input loads on the gpsimd SWDGE queue (parallel transfers);
    # with xt bufs=8 nothing waits in-FIFO, so the collective triggers
    # emitted later are never delayed ----
    wq_all = consts.tile([128, NEC, 128], BF16, tag="wq", name="wq_all")
    nc.sync.dma_start(out=wq_all[:], in_=wqT.rearrange("(c p) f -> p c f", p=128))

    xts = []
    for rt in range(2 * N_QT):
        t = xtp.tile([128, NEC, RT], BF16, tag="xt", bufs=8, name=f"xt_{rt}")
        nc.gpsimd.dma_start(
            out=t[:],
            in_=xT.rearrange("(c p) r -> p c r", p=128)[:, :, rt * RT:(rt + 1) * RT])
        xts.append(t)
        if rt == 0:
            wk_all = consts.tile([128, NEC, 128], BF16, tag="wk", name="wk_all")
            nc.gpsimd.dma_start(
                out=wk_all[:], in_=wkT.rearrange("(c p) f -> p c f", p=128))
            wv_all = consts.tile([128, NEC, 128], BF16, tag="wv", name="wv_all")
            nc.gpsimd.dma_start(
                out=wv_all[:], in_=wvT.rearrange("(c p) f -> p c f", p=128))
            p2_sb = consts.tile([128, 128], BF16, tag="p2", name="p2_sb")
            nc.gpsimd.dma_start(out=p2_sb[:], in_=p2T[:, :])
        if rt == 1:
            cos_sb = consts.tile([128, S], F32, tag="cos", name="cos_sb")
            nc.gpsimd.dma_start(out=cos_sb[:], in_=cosT[:, :])
            sin_sb = consts.tile([128, S], F32, tag="sin", name="sin_sb")
            nc.gpsimd.dma_start(out=sin_sb[:], in_=sinT[:, :])
    # w2: 2 MB, overlaps the batch-0 qkv/attention stretch
    w2_all = consts.tile([128, NEC, E], BF16, tag="w2", name="w2_all")
    nc.gpsimd.dma_start(out=w2_all[:], in_=w2T.rearrange("(c p) f -> p c f", p=128))

    ones_f32 = consts.tile([128, 64], F32, tag="ones32", name="ones_f32")
    nc.vector.memset(ones_f32[:], 1.0)
    ones_rb = consts.tile([1, 64], BF16, tag="onesrb", name="ones_rb")
    nc.vector.tensor_copy(ones_rb[:], ones_f32[0:1, 0:64])
    id_sb = consts.tile([128, 128], F32, tag="idm", name="id_sb")
    make_identity(nc, id_sb[:])

    # A2A buffers, one pair per (batch, half): dest core j's chunk is
    # [2 heads, 65 rows (64 o^T + denominator), 128 s-cols]
    send_d = {(b, hf): dramp.tile([N_CORES, HPC, 65, 128], BF16,
                                  name=f"send{b}{hf}")
              for b in range(B) for hf in ("A", "B")}
    recv_d = {(b, hf): dramp.tile([N_CORES, HPC, 65, 128], BF16,
                                  name=f"recv{b}{hf}")
              for b in range(B) for hf in ("A", "B")}

    qT_sb, kT_sb, v_sb = {}, {}, {}

    def qkv_chains(rt):
        """Return a list of closures, each emitting one matmul chain (+ its
        epilogue) for r-tile rt. Callers dribble these between attention
        steps to keep the in-order PE stream dense but never monolithic."""
        b, st = rt // N_QT, (rt % N_QT) * RT
        xt = xts[rt]

        if b not in qT_sb:
            qT_sb[b] = qkp.tile([128, S], BF16, tag=f"qT{b}", name=f"qT{b}")
            kT_sb[b] = qkp.tile([128, S], BF16, tag=f"kT{b}", name=f"kT{b}")

        def qk_chain(kind, w_all, dst):
            state = {}
            def emit_a():
                acc = ps_qkv.tile([128, RT], F32, tag="qkv",
                                  name=f"{kind}acc{rt}")
                for ec in range(4):
                    nc.tensor.matmul(acc[:], w_all[:, ec, :], xt[:, ec, :],
                                     start=(ec == 0), stop=False)
                state["acc"] = acc
            def emit_b():
                acc = state.pop("acc")
                for ec in range(4, NEC):
                    nc.tensor.matmul(acc[:], w_all[:, ec, :], xt[:, ec, :],
                                     start=False, stop=(ec == NEC - 1))
                raw = rawp.tile([128, RT], BF16, tag="raw",
                                name=f"{kind}raw{rt}")
                # DVE eviction: keep the Scalar engine free for exp, which
                # paces the attention phases these chains dribble into
                nc.vector.tensor_copy(raw[:], acc[:])
                rot = ps_qkv.tile([128, RT], F32, tag="qkv",
                                  name=f"{kind}rot{rt}")
                nc.tensor.matmul(rot[:], p2_sb[:], raw[:], start=True, stop=True)
                t1 = tmpp.tile([128, RT], F32, tag="ropet", name=f"{kind}t1_{rt}")
                nc.vector.tensor_mul(t1[:], raw[:], cos_sb[:, st:st + RT])
                t2 = tmpp.tile([128, RT], F32, tag="ropet", name=f"{kind}t2_{rt}")
                nc.vector.tensor_mul(t2[:], rot[:], sin_sb[:, st:st + RT])
                nc.vector.tensor_add(dst[:, st:st + RT], t1[:], t2[:])
            return [emit_a, emit_b]

        vstate = {}

        def v_head_chain(half):
            # v^T = wv.T @ x computed at full rate (N=512), half the e-chunks
            # per pop; the PE transpose in v_tail_chain flips it back to the
            # [k, hd] layout attn@v needs.
            def emit():
                if half == 0:
                    vacc = ps_qkv.tile([128, RT], F32, tag="qkv",
                                       name=f"vTacc{rt}")
                    vstate["ps"] = vacc
                vacc = vstate["ps"]
                for ec in range(4 * half, 4 * half + 4):
                    nc.tensor.matmul(vacc[:], wv_all[:, ec, :], xt[:, ec, :],
                                     start=(ec == 0), stop=(ec == NEC - 1))
                if half == 1:
                    vts = rawp.tile([128, RT], F32, tag="raw",
                                    name=f"vts{rt}")
                    nc.vector.tensor_copy(vts[:], vstate.pop("ps")[:])
                    vstate["sb"] = vts
            return emit

        def v_tail_chain(pair):
            def emit():
                vts = vstate["sb"]
                for sub in (2 * pair, 2 * pair + 1):
                    vtr = ps_qkv.tile([128, 128], F32, tag="qkv",
                                      name=f"vtr{rt}_{sub}")
                    nc.tensor.transpose(
                        vtr[:], vts[:, sub * 128:(sub + 1) * 128], id_sb[:])
                    kc = (rt % N_QT) * 4 + sub
                    for h in range(HPC):
                        vt = vp.tile([128, 65], BF16, tag=f"v{b}{h}{kc}",
                                     name=f"v{b}{h}{kc}")
                        nc.vector.tensor_copy(vt[:, 0:64],
                                              vtr[:, h * 64:(h + 1) * 64])
                        nc.vector.tensor_copy(vt[:, 64:65], ones_f32[:, 0:1])
                        v_sb[(b, h, kc)] = vt
            return emit

        return qk_chain("q", wq_all, qT_sb[b]) + \
               qk_chain("k", wk_all, kT_sb[b]) + \
               [v_head_chain(0), v_head_chain(1),
                v_tail_chain(0), v_tail_chain(1)]

    def proj_chains(b, hf):
        """Output projection for my 128 rows of (batch b, diagonal half hf).
        The recv load + softmax divide are emitted lazily by the first chain
        so they never precede the collective's emission."""
        state0 = {}
        def get_odv():
            if "odv" not in state0:
                # o^T rows: e-row within source chunk c is h*64+p
                recv_sb = xtp.tile([128, NEC, 128], BF16, tag="recv", bufs=2,
                                   name=f"recv{b}{hf}")
                for h in range(HPC):
                    nc.gpsimd.dma_start(
                        out=recv_sb[h * 64:(h + 1) * 64, :, :],
                        in_=recv_d[(b, hf)][:, h, 0:64, :].rearrange(
                            "c p r -> p c r"))
                # denominator rows, one partition per (source, head)
                dn = smallp.tile([16, 128], BF16, tag="dn", name=f"dn{b}{hf}")
                nc.gpsimd.dma_start(
                    out=dn[:],
                    in_=recv_d[(b, hf)][:, :, 64:65, :].rearrange(
                        "c h p r -> (c h p) r"))
                # all 16 reciprocals in one partition-parallel op (~0.9us)
                rcp16 = smallp.tile([16, 128], BF16, tag="rcp16",
                                    name=f"rcp16{b}{hf}")
                with nc.allow_low_precision(reason="bf16 1/denominator"):
                    nc.vector.reciprocal(rcp16[:], dn[:])
                rcp_dr = dramp.tile([16, 128], BF16, tag="rcpd", bufs=2,
                                    name=f"rcpd{b}{hf}")
                nc.sync.dma_start(out=rcp_dr[:], in_=rcp16[:])
                odv = xtp.tile([128, NEC, 128], BF16, tag="odv", bufs=2,
                               name=f"odv{b}{hf}")
                if b == 1:
                    # tail halves: the attention stream is over, so ps_sps
                    # and the PE are free — broadcast 1/d with one K=1
                    # matmul per head (the replicating-DMA bounce costs
                    # ~10us of strided small reads; this is ~1us)
                    dnr = []
                    for h in range(HPC):
                        t = smallp.tile([1, NEC * 128], BF16, tag=f"dnr{h}",
                                        name=f"dnr{b}{hf}{h}")
                        nc.sync.dma_start(
                            out=t[:],
                            in_=bass.AP(tensor=rcp_dr.tensor,
                                        offset=rcp_dr.offset + h * 128,
                                        ap=[[HPC * 128, N_CORES], [1, 128]]))
                        dnr.append(t)
                    bc_ps = ps_sps.tile([128, NEC * 128], F32, tag="sps",
                                        name=f"bcps{b}{hf}")
                    for h in range(HPC):
                        for half in range(2):
                            sl = slice(half * 512, (half + 1) * 512)
                            nc.tensor.matmul(bc_ps[h * 64:(h + 1) * 64, sl],
                                             ones_rb[:], dnr[h][:, sl],
                                             start=True, stop=True)
                    nc.vector.tensor_mul(
                        odv[:], recv_sb[:],
                        bc_ps[:].rearrange("p (c r) -> p c r", c=NEC))
                else:
                    # mid-stream halves: latency is fully hidden by the
                    # attention stream — broadcast via the DRAM bounce
                    # (partition h*64+rep reads rcp row c*2+h)
                    bcast = smallp.tile([128, NEC, 128], BF16, tag="bcast",
                                        name=f"bcast{b}{hf}")
                    for h in range(HPC):
                        bc_ap = bass.AP(
                            tensor=rcp_dr.tensor,
                            offset=rcp_dr.offset + h * 128,
                            ap=[[0, 64], [HPC * 128, N_CORES], [1, 128]])
                        nc.sync.dma_start(out=bcast[h * 64:(h + 1) * 64, :, :],
                                          in_=bc_ap)
                    nc.vector.tensor_mul(odv[:], recv_sb[:], bcast[:])
                state0["odv"] = odv
            return state0["odv"]
        chains = []
        rblk = {"A": 0, "B": 1}[hf]     # for cores 0-3; host swaps for 4-7
        for ft in range(2):
            state = {}
            def emit_a(ft=ft, state=state):
                odv = get_odv()
                # qkv psum tag: free during attention (projection is done)
                ops = ps_qkv.tile([128, 512], F32, tag="qkv",
                                  name=f"ops{b}_{rblk}_{ft}")
                for ec in range(4):
                    nc.tensor.matmul(
                        ops[:],
                        odv[:, ec, :],
                        w2_all[:, ec, ft * 512:(ft + 1) * 512],
                        start=(ec == 0), stop=False)
                state["ops"] = ops
            def emit_b(ft=ft, state=state):
                odv = get_odv()
                ops = state.pop("ops")
                for ec in range(4, NEC):
                    nc.tensor.matmul(
                        ops[:],
                        odv[:, ec, :],
                        w2_all[:, ec, ft * 512:(ft + 1) * 512],
                        start=False, stop=(ec == NEC - 1))
                ot = tmpp.tile([128, 512], F32, tag="ropet",
                               name=f"ot{b}_{rblk}_{ft}")
                if b == 1 and hf == "A":
                    # kernel tail: exp stream is over, ACT is free
                    nc.scalar.copy(ot[:], ops[:])
                else:
                    # runs during an attention stretch where exp keeps ACT
                    # busy: evict on DVE
                    nc.vector.tensor_copy(ot[:], ops[:])
                ob = 2 * b + rblk
                nc.sync.dma_start(
                    out=out[ob * 128:(ob + 1) * 128,
                            ft * 512:(ft + 1) * 512],
                    in_=ot[:])
            chains.append(emit_a)
            chains.append(emit_b)
        return chains

    def emit_stage(b, qt, avs):
        """Evict the attn@v accumulator (o^T raw + denominator row) straight
        into the A2A send buffer; the divide happens on the receive side."""
        last = (b == B - 1 and qt == N_QT - 1)
        hf = HALF[qt]
        jbase = 4 * (qt % 2)
        for h in range(HPC):
            # evict immediately: releases the PSUM slot so the next q-tile's
            # attn@v never waits
            oraw = smallp.tile([65, QT], BF16, tag="oraw",
                               name=f"oraw{b}{h}{qt}")
            if last:
                nc.scalar.copy(oraw[:], avs[h][:])
            else:
                # exp paces the attention stream: keep evictions off ACT
                nc.vector.tensor_copy(oraw[:], avs[h][:])
            # at the kernel tail these sends gate the final A2A: split them
            # across both HWDGE queues (ACT is idle there)
            eng = nc.scalar if (last and h == 1) else nc.sync
            for jj in range(4):
                eng.dma_start(
                    out=send_d[(b, hf)][jbase + jj, h, :, :],
                    in_=oraw[:, jj * 128:(jj + 1) * 128])

    def emit_attention_batch(b, dribble):
        """All 4 q-tiles of a batch as one rolling pipeline over 64+LAG
        (qt, kc) units: scores+exp lead, attn@v trails by LAG units, the
        staging fires as each q-tile's accumulation completes. One dribble
        chain (qkv projection / output projection) is popped every other
        unit to keep the in-order PE stream dense."""
        scale = 1.0 / math.sqrt(HD)
        NU = N_QT * N_KC
        LAG = 5
        pts = {}
        avs = {}
        for u in range(NU + LAG):
            if u < NU:
                qt, kc = divmod(u, N_KC)
                if kc == 0:
                    avs[qt] = [ps_av.tile([65, QT], F32, tag="av",
                                          name=f"av{b}{h}{qt}")
                               for h in range(HPC)]
                sps = ps_sps.tile([128, 2 * QT], F32, tag="sps",
                                  name=f"s{b}{qt}_{kc}")
                for h in range(HPC):
                    hof = h * 64
                    nc.tensor.matmul(
                        sps[:, h * QT:(h + 1) * QT],
                        kT_sb[b][hof:hof + 64, kc * KC:(kc + 1) * KC],
                        qT_sb[b][hof:hof + 64, qt * QT:(qt + 1) * QT],
                        start=True, stop=True)
                pt = pp.tile([128, 2 * QT], BF16, tag="p", name=f"p{b}{qt}_{kc}")
                nc.scalar.activation(pt[:], sps[:], EXPF, scale=scale)
                pts[u] = pt
            if u >= LAG:
                j = u - LAG
                qt2, kc2 = divmod(j, N_KC)
                for h in range(HPC):
                    nc.tensor.matmul(avs[qt2][h][:], v_sb[(b, h, kc2)][:],
                                     pts[j][:, h * QT:(h + 1) * QT],
                                     start=(kc2 == 0), stop=(kc2 == N_KC - 1))
                del pts[j]
                if kc2 == N_KC - 1:
                    emit_stage(b, qt2, avs.pop(qt2))
                    if qt2 == 2:
                        emit_a2a(b, "B")
            # one chain per two units, ramping up near the end so no
            # backlog remains to run as a monolithic lump afterwards
            if dribble and dribble[0][0] <= u and (
                    u % 2 == 1 or 2 * len(dribble) >= (NU + LAG - u)):
                dribble.pop(0)[1]()

    def emit_a2a(b, hf):
        nc.gpsimd.collective_compute(
            "AllToAll", mybir.AluOpType.bypass,
            replica_groups=[list(range(N_CORES))],
            ins=[send_d[(b, hf)].opt()], outs=[recv_d[(b, hf)].opt()])

    # ---------------- emission ----------------
    for rt in range(N_QT):             # batch-0 projection: pure PE stretch
        for chain in qkv_chains(rt):
            chain()
    # warm the collective path (cold-start ~8us); emitted here so the wait on
    # the gpsimd queue never delays the critical first x/weight loads
    cwu_s = dramp.tile([N_CORES, 8], F32, tag="cwus", name="cwu_s")
    cwu_r = dramp.tile([N_CORES, 8], F32, tag="cwur", name="cwu_r")
    nc.sync.dma_start(out=cwu_s.rearrange("c r -> (c r)")[None, :],
                      in_=ones_f32[0:1, 0:64])
    nc.gpsimd.collective_compute(
        "AllToAll", mybir.AluOpType.bypass,
        replica_groups=[list(range(N_CORES))],
        ins=[cwu_s.opt()], outs=[cwu_r.opt()])

    # batch-0 attention with batch-1 qkv dribbled in; A2A(0,B) fires at 3/4
    dribble = [(1, c) for rt in range(N_QT, 2 * N_QT) for c in qkv_chains(rt)]
    emit_attention_batch(0, dribble)
    for _, chain in dribble:
        chain()
    del dribble[:]
    emit_a2a(0, "A")                   # fires at batch-0 end

    # batch-1 attention: batch-0 projections early (both its A2As have
    # landed). Batch-1's own projections run post-loop: their PE matmuls
    # wait on collectives, and dribbling them would fence the in-order PE
    # stream mid-attention. A2A(1,A) is emitted first so its trigger fires
    # the moment the qt3 sends land.
    dribble = [(2, c) for c in proj_chains(0, "B")]
    dribble += [(14, c) for c in proj_chains(0, "A")]
    emit_attention_batch(1, dribble)
    for _, chain in dribble:
        chain()
    emit_a2a(1, "A")
    for chain in proj_chains(1, "B"):
        chain()
    for chain in proj_chains(1, "A"):
        chain()
    ctx.close()


def _host_prep(x, w1, w2):
    import ml_dtypes
    bf16 = ml_dtypes.bfloat16
    x = np.asarray(x, dtype=np.float32)
    w1 = np.asarray(w1, dtype=np.float32)
    w2 = np.asarray(w2, dtype=np.float32)

    xT = np.ascontiguousarray(x.reshape(R, E).T.astype(bf16))      # [E, R]
    w2T = np.ascontiguousarray(w2.T.astype(bf16))                  # [E, E]

    theta = 1.0 / (BASE ** (np.arange(0, HD, 2, dtype=np.float32) / HD))
    enc = np.arange(S, dtype=np.float32)[:, None] * theta[None, :]
    enc = np.repeat(enc, 2, axis=-1)                      # [s, 64]
    cos1 = np.cos(enc).T.astype(np.float32)               # [64, S]
    sin1 = np.sin(enc).T.astype(np.float32)
    cosT = np.ascontiguousarray(np.concatenate([cos1, cos1], axis=0))
    sinT = np.ascontiguousarray(np.concatenate([sin1, sin1], axis=0))

    m64 = np.zeros((HD, HD), dtype=np.float32)
    for i in range(HD // 2):
        m64[2 * i, 2 * i + 1] = -1.0
        m64[2 * i + 1, 2 * i] = 1.0
    m128 = np.zeros((128, 128), dtype=np.float32)
    m128[:64, :64] = m64
    m128[64:, 64:] = m64
    p2T = np.ascontiguousarray(m128.T.astype(bf16))

    in_maps = []
    for c in range(N_CORES):
        hA, hB = HPC * c, HPC * c + 1
        def rows(base):
            return np.concatenate(
                [w1[base + hA * HD: base + (hA + 1) * HD, :],
                 w1[base + hB * HD: base + (hB + 1) * HD, :]], axis=0)
        in_maps.append({
            "xT": xT,
            "wqT": np.ascontiguousarray(rows(0).T.astype(bf16)),
            "wkT": np.ascontiguousarray(rows(E).T.astype(bf16)),
            "wvT": np.ascontiguousarray(rows(2 * E).T.astype(bf16)),
            "w2T": w2T,
            "cosT": cosT,
            "sinT": sinT,
            "p2T": p2T,
        })
    return in_maps


def kernel(x, w1, w2, _trace=False):
    if "nc" not in _COMPILED:
        _COMPILED["nc"] = _build_nc()
    nc = _COMPILED["nc"]
    in_maps = _host_prep(x, w1, w2)
    res = run_bass_kernel_spmd(nc, in_maps, core_ids=list(range(N_CORES)),
                               trace=_trace)
    _COMPILED["last_result"] = res
    # core c returns [512, E] as four 128-row blocks written per (batch,
    # half): [b0 A, b0 B, b1 A, b1 B]. Half A carries s-rows 128c for cores
    # 0-3 but 1024+128c for cores 4-7 (diagonal halves); B is the opposite.
    full = np.empty((B, S, E), dtype=np.float32)
    for c in range(N_CORES):
        blk = res.results[c]["out"]
        lo, hi = (0, 1) if c < 4 else (1, 0)   # blk index carrying s=128c
        for b in range(B):
            full[b, 128 * c:128 * (c + 1)] = blk[(2 * b + lo) * 128:
                                                 (2 * b + lo + 1) * 128]
            full[b, 1024 + 128 * c:1024 + 128 * (c + 1)] = \
                blk[(2 * b + hi) * 128:(2 * b + hi + 1) * 128]
    return full
